# revision 22
# baseline (speedup 1.0000x reference)
"""Bidirectional Mamba block on 8 Trainium2 NeuronCores.

Sharding: core c in 0..7 handles (branch = c // 4, batch = c % 4) where
branch 0 = fwd, branch 1 = bwd (bwd runs on the time-flipped input; flip
is applied host-side before dispatch and on the partial output after).

Per-core device pipeline (one full mamba branch for one batch element):
  LN -> in_proj (PE, bf16) -> causal depthwise conv (PE, diag matmuls)
     -> silu -> x_proj (PE) -> dt_proj + softplus -> selective scan
     (tensor_tensor_scan per (d_tile, state)) -> y = sum_s C_s*h_s via
     PE identity-matmul PSUM accumulation -> gate with silu(z)
     -> fused (merge_half @ out_w) matmul -> partial output [d_model, L].

Host combines: out = x + part_fwd^T + flip(part_bwd^T) + merge_b.
"""

import math
import os
import sys
from contextlib import ExitStack

import numpy as np

sys.path.insert(0, "/opt/trn_rl_repo")
sys.path.insert(0, "/opt/trn_rl_repo/concourse")

import ml_dtypes  # noqa: E402

import concourse.bass as bass  # noqa: E402
import concourse.tile as tile  # noqa: E402
from concourse import bacc, mybir  # noqa: E402
from concourse.bass_utils import run_bass_kernel_spmd  # noqa: E402
from concourse.masks import make_identity  # noqa: E402

FP32 = mybir.dt.float32
BF16 = mybir.dt.bfloat16
OP = mybir.AluOpType
ACTF = mybir.ActivationFunctionType
BF16_NP = ml_dtypes.bfloat16


class Cfg:
    def __init__(self, L=2048, DM=1024, DI=2048, DS=16, DTR=64, DC=4, TC=512):
        self.L = L      # sequence length
        self.DM = DM    # d_model
        self.DI = DI    # d_inner
        self.DS = DS    # d_state
        self.DTR = DTR  # dt_rank
        self.DC = DC    # d_conv
        self.TC = TC    # time chunk
        self.P = 128
        self.NCH = L // TC          # time chunks
        self.NDH = DI // self.P     # d_inner 128-tiles
        self.NDM = DM // self.P     # d_model 128-tiles
        self.NLT = L // self.P      # L 128-tiles (for LN stats)
        assert L % TC == 0 and DI % 128 == 0 and DM % 128 == 0 and L % 128 == 0
        assert DTR <= 128 and DTR + 2 * DS <= 128


FULL = Cfg()


def build_program(cfg: Cfg, num_devices: int = 8):
    """Build the (shared-across-cores) Bass program."""
    nc = bacc.Bacc(
        "TRN2", target_bir_lowering=False, debug=False, num_devices=num_devices
    )
    P, L = cfg.P, cfg.L

    def ext_in(name, shape, dt=FP32):
        return nc.dram_tensor(name, shape, dt, kind="ExternalInput")

    io = {
        # activations
        "x_ld": ext_in("x_ld", [L, cfg.DM]),          # [L, d_model] fp32
        "x_dl": ext_in("x_dl", [cfg.DM, L]),          # transposed   fp32
        "ln_g": ext_in("ln_g", [cfg.DM, 1]),
        "ln_b": ext_in("ln_b", [cfg.DM, 1]),
        # weights (pre-transposed / pre-cast host side)
        "in_w_pk": ext_in("in_w_pk", [P, 2 * (cfg.DI // P) * cfg.DM], BF16),
        "conv_w": ext_in("conv_w", [cfg.DI, cfg.DC]),
        "conv_dg_pk": ext_in(
            "conv_dg_pk", [P, (cfg.DI // P) * cfg.DC * P], BF16),
        "conv_b": ext_in("conv_b", [cfg.DI, 1]),
        "xproj_wT": ext_in("xproj_wT", [cfg.DI, cfg.DTR + 2 * cfg.DS], BF16),
        "dt_wT": ext_in("dt_wT", [cfg.DTR, cfg.DI], BF16),
        "dt_b": ext_in("dt_b", [cfg.DI, 1]),
        "A_neg": ext_in("A_neg", [cfg.DI, cfg.DS]),   # -exp(A_log) fp32
        "D_vec": ext_in("D_vec", [cfg.DI, 1]),
        "w_comb_pk": ext_in("w_comb_pk", [P, (cfg.DM // P) * cfg.DI], BF16),
    }
    out = nc.dram_tensor("part_out", [cfg.DM, L], FP32, kind="ExternalOutput")
    # internal DRAM scratch
    scratch = {
        "mu_d": nc.dram_tensor("mu_d", [L, 1], FP32),
        "rstd_d": nc.dram_tensor("rstd_d", [L, 1], FP32),
        "bc_d": nc.dram_tensor("bc_d", [2 * cfg.DS, L], BF16),
        "z_d": nc.dram_tensor("z_d", [cfg.DI, L], BF16),
    }

    with tile.TileContext(nc) as tc:
        with ExitStack() as ctx:
            _body(ctx, tc, cfg, io, out, scratch)
    nc.compile()
    return nc


def _body(ctx, tc, cfg, io, out_d, scratch):
    nc = tc.nc
    P, L, TC, DS, DC = cfg.P, cfg.L, cfg.TC, cfg.DS, cfg.DC
    NCH, NDH, NDM = cfg.NCH, cfg.NDH, cfg.NDM
    NLT, DTR = cfg.NLT, cfg.DTR
    CW = TC + DC - 1  # conv input window per chunk in the xz store
    NPJ = DTR + 2 * DS
    mu_d, rstd_d, bc_d, z_d = (scratch["mu_d"], scratch["rstd_d"],
                               scratch["bc_d"], scratch["z_d"])
    G4 = 4 if NDH % 4 == 0 else (2 if NDH % 2 == 0 else 1)

    # ---------------- persistent pools / tiles ----------------
    const_p = ctx.enter_context(tc.tile_pool(name="const", bufs=1))
    big_p = ctx.enter_context(tc.tile_pool(name="big", bufs=1))

    ident = const_p.tile([P, P], BF16, tag="ident")
    make_identity(nc, ident[:])

    # small per-channel columns packed into one tile:
    # [0:NDH*DC conv_w][NDH conv_b][NDH dt_b][NDH D][NDM g][NDM b][1 eps]
    ncc = NDH * DC + 3 * NDH + 2 * NDM + 1
    cols = const_p.tile([P, ncc], FP32, tag="cols")
    o_cw, o_cb, o_db, o_dv = 0, NDH * DC, NDH * DC + NDH, NDH * DC + 2 * NDH
    o_g = NDH * DC + 3 * NDH
    o_b = o_g + NDM
    o_eps = o_b + NDM
    conv_w_c = lambda k, t: cols[:, o_cw + k * DC + t:o_cw + k * DC + t + 1]
    conv_b_c = lambda k: cols[:, o_cb + k:o_cb + k + 1]
    dt_b_c = lambda k: cols[:, o_db + k:o_db + k + 1]
    d_c = lambda k: cols[:, o_dv + k:o_dv + k + 1]
    g_c = lambda k: cols[:, o_g + k:o_g + k + 1]
    b_c = lambda k: cols[:, o_b + k:o_b + k + 1]
    eps_c = cols[:, o_eps:o_eps + 1]
    nc.vector.memset(eps_c, 1e-5)
    for k in range(NDH):
        r = slice(k * P, (k + 1) * P)
        nc.sync.dma_start(cols[:, o_cw + k * DC:o_cw + (k + 1) * DC],
                          io["conv_w"][r, :])
        nc.sync.dma_start(conv_b_c(k), io["conv_b"][r, :])
        nc.sync.dma_start(dt_b_c(k), io["dt_b"][r, :])
        nc.sync.dma_start(d_c(k), io["D_vec"][r, :])
    for k in range(NDM):
        r = slice(k * P, (k + 1) * P)
        nc.sync.dma_start(g_c(k), io["ln_g"][r, :])
        nc.sync.dma_start(b_c(k), io["ln_b"][r, :])

    a_sb = const_p.tile([P, NDH * DS], FP32, tag="aneg")
    for k in range(NDH):
        nc.sync.dma_start(a_sb[:, k * DS:(k + 1) * DS],
                          io["A_neg"][k * P:(k + 1) * P, :])

    # x_proj / dt_proj weights resident, bf16
    xprj_sb = const_p.tile([P, NDH * NPJ], BF16, tag="xprj")
    for k in range(NDH):
        nc.sync.dma_start(
            xprj_sb[:, k * NPJ:(k + 1) * NPJ], io["xproj_wT"][k * P:(k + 1) * P, :]
        )
    dtw_sb = const_p.tile([DTR, cfg.DI], BF16, tag="dtw")
    nc.sync.dma_start(dtw_sb[:], io["dt_wT"][:, :])

    # ---------------- phase 1: LayerNorm statistics ----------------
    with tc.tile_pool(name="ln", bufs=2) as ln_p:
        for lt in range(NLT):
            r = slice(lt * P, (lt + 1) * P)
            xt = ln_p.tile([P, cfg.DM], FP32, tag="x")
            nc.sync.dma_start(xt[:], io["x_ld"][r, :])
            s1 = ln_p.tile([P, 1], FP32, tag="s1")
            nc.vector.reduce_sum(s1[:], xt[:], axis=mybir.AxisListType.X)
            negmu = ln_p.tile([P, 1], FP32, tag="negmu")
            nc.scalar.mul(negmu[:], s1[:], -1.0 / cfg.DM)
            mu = ln_p.tile([P, 1], FP32, tag="mu")
            nc.scalar.mul(mu[:], s1[:], 1.0 / cfg.DM)
            sq = ln_p.tile([P, cfg.DM], FP32, tag="sq")
            ss = ln_p.tile([P, 1], FP32, tag="ss")
            nc.scalar.activation(sq[:], xt[:], ACTF.Square, bias=negmu[:],
                                 scale=1.0, accum_out=ss[:])
            # rstd = exp(-0.5 * ln(var + eps)); keeps ACT in the exp/ln table
            lv = ln_p.tile([P, 1], FP32, tag="lv")
            nc.scalar.activation(lv[:], ss[:], ACTF.Ln, bias=eps_c,
                                 scale=1.0 / cfg.DM)
            rstd = ln_p.tile([P, 1], FP32, tag="rstd")
            nc.scalar.activation(rstd[:], lv[:], ACTF.Exp, scale=-0.5)
            nc.sync.dma_start(mu_d[r, :], mu[:])
            nc.sync.dma_start(rstd_d[r, :], rstd[:])

    # ---------------- persistent chunk-state tiles ----------------
    xz_xi = big_p.tile([P, NDH * CW], BF16, tag="xz_xi")
    xiT = big_p.tile([P, NDH * TC], BF16, tag="xiT")
    delta = big_p.tile([P, NDH * TC], BF16, tag="delta")
    b_big = big_p.tile([P, DS, TC], BF16, tag="b_big")
    c_big = big_p.tile([P, DS, TC], BF16, tag="c_big")
    h_big = big_p.tile([P, DS, TC], BF16, tag="h_big")
    ygate = big_p.tile([P, NDH * TC], BF16, tag="ygate")
    carry = big_p.tile([P, NDH * DS], FP32, tag="carry")
    xnT = big_p.tile([P, NDM * TC], BF16, tag="xnT")

    for k in range(NDH):  # zero the conv left-pad for chunk 0
        nc.vector.memset(xz_xi[:, k * CW:k * CW + DC - 1], 0.0)

    wi_p = ctx.enter_context(tc.tile_pool(name="wi", bufs=2))
    wo_p = ctx.enter_context(tc.tile_pool(name="wo", bufs=2))
    mm_ps = ctx.enter_context(
        tc.tile_pool(name="mmps", bufs=3, space=bass.MemorySpace.PSUM))
    y_ps_p = ctx.enter_context(
        tc.tile_pool(name="yps", bufs=2, space=bass.MemorySpace.PSUM))
    o_ps_p = ctx.enter_context(
        tc.tile_pool(name="ops", bufs=2, space=bass.MemorySpace.PSUM))
    sc_p = ctx.enter_context(tc.tile_pool(name="scan", bufs=3))
    t16_p = ctx.enter_context(tc.tile_pool(name="t16", bufs=2))
    ev_p = ctx.enter_context(tc.tile_pool(name="evac", bufs=2))
    bat_p = ctx.enter_context(tc.tile_pool(name="bat", bufs=2))
    lnc_p = ctx.enter_context(tc.tile_pool(name="lnc", bufs=1))

    for ch in range(NCH):
        tsl = slice(ch * TC, (ch + 1) * TC)

        # ---- LayerNorm apply (chunked): xnT[k] = ((xT - mu) * rstd) * g + b
        mus = lnc_p.tile([P, 2 * TC], FP32, tag="mus")
        nc.sync.dma_start(
            mus[:, 0:TC],
            mu_d.ap().rearrange("l one -> one l")[0:1, tsl].partition_broadcast(P))
        nc.sync.dma_start(
            mus[:, TC:2 * TC],
            rstd_d.ap().rearrange("l one -> one l")[0:1, tsl].partition_broadcast(P))
        for k in range(NDM):
            xtT = ev_p.tile([P, TC], FP32, tag="softp")
            nc.sync.dma_start(xtT[:], io["x_dl"][k * P:(k + 1) * P, tsl])
            nc.vector.tensor_sub(xtT[:], xtT[:], mus[:, 0:TC])
            nc.vector.tensor_mul(xtT[:], xtT[:], mus[:, TC:2 * TC])
            nc.scalar.activation(xnT[:, k * TC:(k + 1) * TC], xtT[:],
                                 ACTF.Identity, bias=b_c(k), scale=g_c(k))

        # ---- in_proj: xi half -> xz_xi store; z half -> silu'd later via DRAM
        zb = None
        for m in range(2 * NDH):
            wt = wi_p.tile([P, NDM * P], BF16, tag="w_in")
            nc.sync.dma_start(
                wt[:], io["in_w_pk"][:, m * NDM * P:(m + 1) * NDM * P])
            ps = mm_ps.tile([P, TC], FP32, tag="mm")
            for k in range(NDM):
                nc.tensor.matmul(ps[:], wt[:, k * P:(k + 1) * P],
                                 xnT[:, k * TC:(k + 1) * TC],
                                 start=(k == 0), stop=(k == NDM - 1))
            if m < NDH:
                nc.scalar.copy(xz_xi[:, m * CW + DC - 1:m * CW + DC - 1 + TC], ps[:])
            else:
                # store silu(z) = z * sigmoid(z); batch sigmoid over 4 m-tiles
                # to keep the ACT table from thrashing between exp and sigmoid
                j = (m - NDH) % G4
                if j == 0:
                    zb = bat_p.tile([P, G4, TC], BF16, tag="bat")
                nc.scalar.copy(zb[:, j, :], ps[:])
                if j == G4 - 1:
                    g4 = (m - NDH) // G4
                    zg = bat_p.tile([P, G4, TC], BF16, tag="sig")
                    nc.scalar.activation(zg[:], zb[:], ACTF.Sigmoid)
                    sz = bat_p.tile([P, G4, TC], BF16, tag="slu")
                    nc.vector.tensor_mul(sz[:], zb[:], zg[:])
                    nc.sync.dma_start(
                        z_d[g4 * G4 * P:(g4 + 1) * G4 * P, tsl]
                        .rearrange("(j p) t -> p j t", p=P), sz[:])

        # ---- causal depthwise conv (PE diag matmuls) + bias + silu -> xiT
        cvb = None
        for k in range(NDH):
            dg = wi_p.tile([P, DC * P], BF16, tag="w_dg")
            nc.sync.dma_start(
                dg[:], io["conv_dg_pk"][:, k * DC * P:(k + 1) * DC * P])
            ps = mm_ps.tile([P, TC], FP32, tag="mm")
            for t in range(DC):
                nc.tensor.matmul(
                    ps[:], dg[:, t * P:(t + 1) * P],
                    xz_xi[:, k * CW + t:k * CW + t + TC],
                    start=(t == 0), stop=(t == DC - 1))
            cv = ev_p.tile([P, TC], BF16, tag="cv")
            nc.scalar.activation(cv[:], ps[:], ACTF.Identity,
                                 bias=conv_b_c(k))
            cg = ev_p.tile([P, TC], BF16, tag="cg")
            nc.scalar.activation(cg[:], ps[:], ACTF.Sigmoid,
                                 bias=conv_b_c(k))
            nc.vector.tensor_mul(xiT[:, k * TC:(k + 1) * TC], cv[:], cg[:])
            # carry last DC-1 input cols to the head for the next chunk
            if ch + 1 < NCH:
                nc.vector.tensor_copy(xz_xi[:, k * CW:k * CW + DC - 1],
                                      xz_xi[:, k * CW + TC:k * CW + TC + DC - 1])

        # ---- x_proj -> proj[NPJ, TC] -> dtT, B, C
        psx = mm_ps.tile([NPJ, TC], FP32, tag="mm")
        for k in range(NDH):
            nc.tensor.matmul(psx[:], xprj_sb[:, k * NPJ:(k + 1) * NPJ],
                             xiT[:, k * TC:(k + 1) * TC],
                             start=(k == 0), stop=(k == NDH - 1))
        dtT = bat_p.tile([DTR, TC], BF16, tag="dtT")
        nc.scalar.copy(dtT[:], psx[0:DTR, :])
        bc_sb = bat_p.tile([2 * DS, TC], BF16, tag="bc")
        nc.scalar.copy(bc_sb[:], psx[DTR:NPJ, :])
        nc.sync.dma_start(bc_d[:, tsl], bc_sb[:])
        nc.sync.dma_start(b_big[:, :, :],
                          bc_d[0:DS, tsl].partition_broadcast(P))
        nc.sync.dma_start(c_big[:, :, :],
                          bc_d[DS:2 * DS, tsl].partition_broadcast(P))

        # ---- dt_proj + softplus -> delta (bf16)
        # softplus(x) = ln(1 + exp(x)); x <= ~2 here so exp never overflows
        for k in range(NDH):
            psd = mm_ps.tile([P, TC], FP32, tag="mm")
            nc.tensor.matmul(psd[:], dtw_sb[:, k * P:(k + 1) * P], dtT[:],
                             start=True, stop=True)
            ex = ev_p.tile([P, TC], BF16, tag="softp")
            nc.scalar.activation(ex[:], psd[:], ACTF.Exp, bias=dt_b_c(k))
            nc.scalar.activation(delta[:, k * TC:(k + 1) * TC], ex[:],
                                 ACTF.Ln, bias=1.0)

        # ---- selective scan per (d_tile, state) + y accumulation on PE
        for k in range(NDH):
            ksl = slice(k * TC, (k + 1) * TC)
            dx = t16_p.tile([P, TC], BF16, tag="dx")
            nc.vector.tensor_mul(dx[:], delta[:, ksl], xiT[:, ksl])
            xid = t16_p.tile([P, TC], BF16, tag="xid")
            nc.vector.tensor_scalar_mul(xid[:], xiT[:, ksl], d_c(k))
            yp = y_ps_p.tile([P, TC], FP32, tag="y")
            nc.tensor.matmul(yp[:], ident[:], xid[:], start=True, stop=False)
            for s in range(DS):
                av = sc_p.tile([P, TC], FP32, tag="a")
                nc.scalar.activation(av[:], delta[:, ksl], ACTF.Exp,
                                     scale=a_sb[:, k * DS + s:k * DS + s + 1])
                uv = sc_p.tile([P, TC], BF16, tag="u")
                if s < DS // 2:  # split u-multiplies across GPSIMD and DVE
                    nc.gpsimd.tensor_mul(uv[:], dx[:], b_big[:, s, :])
                else:
                    nc.vector.tensor_mul(uv[:], dx[:], b_big[:, s, :])
                init = 0.0 if ch == 0 else carry[:, k * DS + s:k * DS + s + 1]
                nc.vector.tensor_tensor_scan(h_big[:, s, :], av[:], uv[:],
                                             init, op0=OP.mult, op1=OP.add)
                if s % 2 == 1:  # hC for (s-1, s) in one paired multiply
                    hc = t16_p.tile([P, 2, TC], BF16, tag="hc")
                    nc.vector.tensor_mul(hc[:], h_big[:, s - 1:s + 1, :],
                                         c_big[:, s - 1:s + 1, :])
                    nc.tensor.matmul(yp[:], ident[:], hc[:, 0, :], start=False,
                                     stop=False)
                    nc.tensor.matmul(yp[:], ident[:], hc[:, 1, :], start=False,
                                     stop=(s == DS - 1))
            if ch + 1 < NCH:  # save carries h[:, -1] for all s of this d-tile
                nc.vector.tensor_copy(carry[:, k * DS:(k + 1) * DS],
                                      h_big[:, :, TC - 1])
            # gate: y * silu(z)  (z_d already holds silu(z))
            zl = t16_p.tile([P, TC], BF16, tag="zl")
            nc.sync.dma_start(zl[:], z_d[k * P:(k + 1) * P, tsl])
            nc.vector.tensor_mul(ygate[:, ksl], yp[:], zl[:])

        # ---- fused output projection
        for m in range(NDM):
            wt = wo_p.tile([P, NDH * P], BF16, tag="w_out")
            nc.sync.dma_start(
                wt[:], io["w_comb_pk"][:, m * NDH * P:(m + 1) * NDH * P])
            po = o_ps_p.tile([P, TC], FP32, tag="o")
            for k in range(NDH):
                nc.tensor.matmul(po[:], wt[:, k * P:(k + 1) * P],
                                 ygate[:, k * TC:(k + 1) * TC],
                                 start=(k == 0), stop=(k == NDH - 1))
            ot = ev_p.tile([P, TC], FP32, tag="out")
            nc.scalar.copy(ot[:], po[:])
            nc.sync.dma_start(out_d[m * P:(m + 1) * P, tsl], ot[:])


# ------------------------------------------------------------------
# host side
# ------------------------------------------------------------------

def _prep_core_inputs(cfg, xb, ln_g, ln_b, w):
    """xb: [L, DM] fp32 (already flipped for bwd). w: per-branch weights dict."""
    d = {
        "x_ld": np.ascontiguousarray(xb, np.float32),
        "x_dl": np.ascontiguousarray(xb.T, np.float32),
        "ln_g": np.ascontiguousarray(ln_g.reshape(-1, 1), np.float32),
        "ln_b": np.ascontiguousarray(ln_b.reshape(-1, 1), np.float32),
    }
    d.update(w)
    return d


def _prep_branch_weights(cfg, in_w, conv_w, conv_b, xproj_w, dt_w, dt_b,
                         A_log, D, out_w, merge_half):
    w_comb = merge_half.astype(np.float64) @ out_w.astype(np.float64)  # [DM, DI]
    P = cfg.P

    def pack_lhsT(w):  # w: [M, K] -> [P, (M//P)*K]; block m holds w[mP:(m+1)P].T
        M, Kd = w.shape
        blocks = [w[m * P:(m + 1) * P, :].reshape(P, Kd // P, P)
                  .transpose(2, 1, 0).reshape(P, Kd)
                  for m in range(M // P)]
        return np.ascontiguousarray(np.concatenate(blocks, axis=1), BF16_NP)

    cw = np.asarray(conv_w, np.float32)
    DI, DC = cw.shape
    dg = np.zeros((DI // P, DC, P, P), np.float32)
    idx = np.arange(P)
    for k in range(DI // P):
        for t in range(DC):
            dg[k, t, idx, idx] = cw[k * P:(k + 1) * P, t]
    dg_pk = np.ascontiguousarray(
        dg.transpose(2, 0, 1, 3).reshape(P, (DI // P) * DC * P), BF16_NP)

    return {
        "in_w_pk": pack_lhsT(np.asarray(in_w, np.float32)),
        "conv_dg_pk": dg_pk,
        "conv_w": np.ascontiguousarray(conv_w, np.float32),
        "conv_b": np.ascontiguousarray(conv_b.reshape(-1, 1), np.float32),
        "xproj_wT": np.ascontiguousarray(xproj_w.T, BF16_NP),
        "dt_wT": np.ascontiguousarray(dt_w.T, BF16_NP),
        "dt_b": np.ascontiguousarray(dt_b.reshape(-1, 1), np.float32),
        "A_neg": np.ascontiguousarray(-np.exp(A_log), np.float32),
        "D_vec": np.ascontiguousarray(D.reshape(-1, 1), np.float32),
        "w_comb_pk": pack_lhsT(w_comb.astype(np.float32)),
    }


_PROG_CACHE = {}


def _get_program(cfg: Cfg, num_devices: int):
    key = (cfg.L, cfg.DM, cfg.DI, cfg.DS, cfg.DTR, cfg.DC, cfg.TC, num_devices)
    if key not in _PROG_CACHE:
        _PROG_CACHE[key] = build_program(cfg, num_devices)
    return _PROG_CACHE[key]


def kernel(x, ln_g, ln_b, merge_w, merge_b,
           fwd_in_w, fwd_conv_w, fwd_conv_b, fwd_xproj_w, fwd_dt_w, fwd_dt_b,
           fwd_A_log, fwd_D, fwd_out_w,
           bwd_in_w, bwd_conv_w, bwd_conv_b, bwd_xproj_w, bwd_dt_w, bwd_dt_b,
           bwd_A_log, bwd_D, bwd_out_w):
    cfg = FULL
    x = np.asarray(x, np.float32)
    B = x.shape[0]
    assert x.shape == (B, cfg.L, cfg.DM) and B == 4

    nc = _get_program(cfg, 8)

    fw = _prep_branch_weights(cfg, fwd_in_w, fwd_conv_w, fwd_conv_b,
                              fwd_xproj_w, fwd_dt_w, fwd_dt_b, fwd_A_log,
                              fwd_D, fwd_out_w, np.asarray(merge_w)[:, :cfg.DM])
    bw = _prep_branch_weights(cfg, bwd_in_w, bwd_conv_w, bwd_conv_b,
                              bwd_xproj_w, bwd_dt_w, bwd_dt_b, bwd_A_log,
                              bwd_D, bwd_out_w, np.asarray(merge_w)[:, cfg.DM:])

    in_maps = []
    for c in range(8):
        br, b = divmod(c, 4)
        xb = x[b] if br == 0 else x[b, ::-1]
        in_maps.append(_prep_core_inputs(cfg, xb, np.asarray(ln_g),
                                         np.asarray(ln_b), fw if br == 0 else bw))

    global _last_in_maps
    _last_in_maps = in_maps
    res = run_bass_kernel_spmd(nc, in_maps, list(range(8)))
    parts = [r["part_out"] for r in res.results]  # [DM, L] each

    out = x.copy()
    for b in range(4):
        out[b] += parts[b].T
        out[b] += parts[4 + b].T[::-1]
    out += np.asarray(merge_b, np.float32)
    return out


# revision 23
# speedup vs baseline: 32.7719x; 32.7719x over previous
"""Bidirectional Mamba block on 8 Trainium2 NeuronCores.

Sharding: core c in 0..7 handles (branch = c // 4, batch = c % 4) where
branch 0 = fwd, branch 1 = bwd (bwd runs on the time-flipped input; flip
is applied host-side before dispatch and on the partial output after).

Per-core device pipeline (one full mamba branch for one batch element):
  LN -> in_proj (PE, bf16) -> causal depthwise conv (PE, diag matmuls)
     -> silu -> x_proj (PE) -> dt_proj + softplus -> selective scan
     (tensor_tensor_scan per (d_tile, state)) -> y = sum_s C_s*h_s via
     PE identity-matmul PSUM accumulation -> gate with silu(z)
     -> fused (merge_half @ out_w) matmul -> partial output [d_model, L].

Host combines: out = x + part_fwd^T + flip(part_bwd^T) + merge_b.
"""

import math
import os
import sys
from contextlib import ExitStack

import numpy as np

sys.path.insert(0, "/opt/trn_rl_repo")
sys.path.insert(0, "/opt/trn_rl_repo/concourse")

import ml_dtypes  # noqa: E402

import concourse.bass as bass  # noqa: E402
import concourse.tile as tile  # noqa: E402
from concourse import bacc, mybir  # noqa: E402
from concourse.bass_utils import run_bass_kernel_spmd  # noqa: E402
from concourse.masks import make_identity  # noqa: E402

FP32 = mybir.dt.float32
BF16 = mybir.dt.bfloat16
OP = mybir.AluOpType
ACTF = mybir.ActivationFunctionType
BF16_NP = ml_dtypes.bfloat16


class Cfg:
    def __init__(self, L=2048, DM=1024, DI=2048, DS=16, DTR=64, DC=4, TC=512):
        self.L = L      # sequence length
        self.DM = DM    # d_model
        self.DI = DI    # d_inner
        self.DS = DS    # d_state
        self.DTR = DTR  # dt_rank
        self.DC = DC    # d_conv
        self.TC = TC    # time chunk
        self.P = 128
        self.NCH = L // TC          # time chunks
        self.NDH = DI // self.P     # d_inner 128-tiles
        self.NDM = DM // self.P     # d_model 128-tiles
        self.NLT = L // self.P      # L 128-tiles (for LN stats)
        assert L % TC == 0 and DI % 128 == 0 and DM % 128 == 0 and L % 128 == 0
        assert DTR <= 128 and DTR + 2 * DS <= 128


FULL = Cfg()


def build_program(cfg: Cfg, num_devices: int = 8):
    """Build the (shared-across-cores) Bass program."""
    nc = bacc.Bacc(
        "TRN2", target_bir_lowering=False, debug=False, num_devices=num_devices
    )
    P, L = cfg.P, cfg.L

    def ext_in(name, shape, dt=FP32):
        return nc.dram_tensor(name, shape, dt, kind="ExternalInput")

    io = {
        # activations
        "x_ld": ext_in("x_ld", [L, cfg.DM]),          # [L, d_model] fp32
        "x_dl": ext_in("x_dl", [cfg.DM, L]),          # transposed   fp32
        "ln_g": ext_in("ln_g", [cfg.DM, 1]),
        "ln_b": ext_in("ln_b", [cfg.DM, 1]),
        # weights (pre-transposed / pre-cast host side)
        "in_w_pk": ext_in("in_w_pk", [P, 2 * (cfg.DI // P) * cfg.DM], BF16),
        "conv_w": ext_in("conv_w", [cfg.DI, cfg.DC]),
        "conv_dg_pk": ext_in(
            "conv_dg_pk", [P, (cfg.DI // P) * cfg.DC * P], BF16),
        "conv_b": ext_in("conv_b", [cfg.DI, 1]),
        "xproj_wT": ext_in("xproj_wT", [cfg.DI, cfg.DTR + 2 * cfg.DS], BF16),
        "dt_wT": ext_in("dt_wT", [cfg.DTR, cfg.DI], BF16),
        "dt_b": ext_in("dt_b", [cfg.DI, 1]),
        "A_neg": ext_in("A_neg", [cfg.DI, cfg.DS]),   # -exp(A_log) fp32
        "D_vec": ext_in("D_vec", [cfg.DI, 1]),
        "w_comb_pk": ext_in("w_comb_pk", [P, (cfg.DM // P) * cfg.DI], BF16),
    }
    out = nc.dram_tensor("part_out", [cfg.DM, L], FP32, kind="ExternalOutput")
    # internal DRAM scratch
    scratch = {
        "mu_d": nc.dram_tensor("mu_d", [L, 1], FP32),
        "rstd_d": nc.dram_tensor("rstd_d", [L, 1], FP32),
        "bc_d": nc.dram_tensor("bc_d", [2 * cfg.DS, L], BF16),
        "z_d": nc.dram_tensor("z_d", [cfg.DI, L], BF16),
    }

    with tile.TileContext(nc) as tc:
        with ExitStack() as ctx:
            _body(ctx, tc, cfg, io, out, scratch)
    nc.compile()
    return nc


def _body(ctx, tc, cfg, io, out_d, scratch):
    nc = tc.nc
    P, L, TC, DS, DC = cfg.P, cfg.L, cfg.TC, cfg.DS, cfg.DC
    NCH, NDH, NDM = cfg.NCH, cfg.NDH, cfg.NDM
    NLT, DTR = cfg.NLT, cfg.DTR
    CW = TC + DC - 1  # conv input window per chunk in the xz store
    NPJ = DTR + 2 * DS
    mu_d, rstd_d, bc_d, z_d = (scratch["mu_d"], scratch["rstd_d"],
                               scratch["bc_d"], scratch["z_d"])
    G4 = 4 if NDH % 4 == 0 else (2 if NDH % 2 == 0 else 1)

    # ---------------- persistent pools / tiles ----------------
    const_p = ctx.enter_context(tc.tile_pool(name="const", bufs=1))
    big_p = ctx.enter_context(tc.tile_pool(name="big", bufs=1))

    ident = const_p.tile([P, P], BF16, tag="ident")
    make_identity(nc, ident[:])

    # small per-channel columns packed into one tile:
    # [0:NDH*DC conv_w][NDH conv_b][NDH dt_b][NDH D][NDM g][NDM b][1 eps]
    ncc = NDH * DC + 3 * NDH + 2 * NDM + 1
    cols = const_p.tile([P, ncc], FP32, tag="cols")
    o_cw, o_cb, o_db, o_dv = 0, NDH * DC, NDH * DC + NDH, NDH * DC + 2 * NDH
    o_g = NDH * DC + 3 * NDH
    o_b = o_g + NDM
    o_eps = o_b + NDM
    conv_w_c = lambda k, t: cols[:, o_cw + k * DC + t:o_cw + k * DC + t + 1]
    conv_b_c = lambda k: cols[:, o_cb + k:o_cb + k + 1]
    dt_b_c = lambda k: cols[:, o_db + k:o_db + k + 1]
    d_c = lambda k: cols[:, o_dv + k:o_dv + k + 1]
    g_c = lambda k: cols[:, o_g + k:o_g + k + 1]
    b_c = lambda k: cols[:, o_b + k:o_b + k + 1]
    eps_c = cols[:, o_eps:o_eps + 1]
    nc.vector.memset(eps_c, 1e-5)
    for k in range(NDH):
        r = slice(k * P, (k + 1) * P)
        nc.sync.dma_start(cols[:, o_cw + k * DC:o_cw + (k + 1) * DC],
                          io["conv_w"][r, :])
        nc.sync.dma_start(conv_b_c(k), io["conv_b"][r, :])
        nc.sync.dma_start(dt_b_c(k), io["dt_b"][r, :])
        nc.sync.dma_start(d_c(k), io["D_vec"][r, :])
    for k in range(NDM):
        r = slice(k * P, (k + 1) * P)
        nc.sync.dma_start(g_c(k), io["ln_g"][r, :])
        nc.sync.dma_start(b_c(k), io["ln_b"][r, :])

    # ---------------- phase 1: LayerNorm statistics ----------------
    with tc.tile_pool(name="ln", bufs=2) as ln_p:
        for lt in range(NLT):
            r = slice(lt * P, (lt + 1) * P)
            xt = ln_p.tile([P, cfg.DM], FP32, tag="x")
            nc.sync.dma_start(xt[:], io["x_ld"][r, :])
            s1 = ln_p.tile([P, 1], FP32, tag="s1")
            nc.vector.reduce_sum(s1[:], xt[:], axis=mybir.AxisListType.X)
            negmu = ln_p.tile([P, 1], FP32, tag="negmu")
            nc.scalar.mul(negmu[:], s1[:], -1.0 / cfg.DM)
            mu = ln_p.tile([P, 1], FP32, tag="mu")
            nc.scalar.mul(mu[:], s1[:], 1.0 / cfg.DM)
            sq = ln_p.tile([P, cfg.DM], FP32, tag="sq")
            ss = ln_p.tile([P, 1], FP32, tag="ss")
            nc.scalar.activation(sq[:], xt[:], ACTF.Square, bias=negmu[:],
                                 scale=1.0, accum_out=ss[:])
            # rstd = exp(-0.5 * ln(var + eps)); keeps ACT in the exp/ln table
            lv = ln_p.tile([P, 1], FP32, tag="lv")
            nc.scalar.activation(lv[:], ss[:], ACTF.Ln, bias=eps_c,
                                 scale=1.0 / cfg.DM)
            rstd = ln_p.tile([P, 1], FP32, tag="rstd")
            nc.scalar.activation(rstd[:], lv[:], ACTF.Exp, scale=-0.5)
            nc.sync.dma_start(mu_d[r, :], mu[:])
            nc.sync.dma_start(rstd_d[r, :], rstd[:])

    a_sb = const_p.tile([P, NDH * DS], FP32, tag="aneg")
    for k in range(NDH):
        nc.sync.dma_start(a_sb[:, k * DS:(k + 1) * DS],
                          io["A_neg"][k * P:(k + 1) * P, :])

    # x_proj / dt_proj weights resident, bf16
    xprj_sb = const_p.tile([P, NDH * NPJ], BF16, tag="xprj")
    for k in range(NDH):
        nc.sync.dma_start(
            xprj_sb[:, k * NPJ:(k + 1) * NPJ], io["xproj_wT"][k * P:(k + 1) * P, :]
        )
    dtw_sb = const_p.tile([DTR, cfg.DI], BF16, tag="dtw")
    nc.sync.dma_start(dtw_sb[:], io["dt_wT"][:, :])

    # ---------------- persistent chunk-state tiles ----------------
    xz_xi = big_p.tile([P, NDH * CW], BF16, tag="xz_xi")
    xiT = big_p.tile([P, NDH * TC], BF16, tag="xiT")
    delta = big_p.tile([P, NDH * TC], BF16, tag="delta")
    b_big = big_p.tile([P, DS, TC], BF16, tag="b_big")
    c_big = big_p.tile([P, DS, TC], BF16, tag="c_big")
    h_big = big_p.tile([P, DS, TC], BF16, tag="h_big")
    ygate = big_p.tile([P, NDH * TC], BF16, tag="ygate")
    carry = big_p.tile([P, NDH * DS], FP32, tag="carry")
    xnT = big_p.tile([P, NDM * TC], BF16, tag="xnT")

    for k in range(NDH):  # zero the conv left-pad for chunk 0
        nc.vector.memset(xz_xi[:, k * CW:k * CW + DC - 1], 0.0)

    wi_p = ctx.enter_context(tc.tile_pool(name="wi", bufs=2))
    wo_p = ctx.enter_context(tc.tile_pool(name="wo", bufs=2))
    mm_ps = ctx.enter_context(
        tc.tile_pool(name="mmps", bufs=3, space=bass.MemorySpace.PSUM))
    y_ps_p = ctx.enter_context(
        tc.tile_pool(name="yps", bufs=2, space=bass.MemorySpace.PSUM))
    o_ps_p = ctx.enter_context(
        tc.tile_pool(name="ops", bufs=2, space=bass.MemorySpace.PSUM))
    sc_p = ctx.enter_context(tc.tile_pool(name="scan", bufs=3))
    t16_p = ctx.enter_context(tc.tile_pool(name="t16", bufs=2))
    ev_p = ctx.enter_context(tc.tile_pool(name="evac", bufs=2))
    bat_p = ctx.enter_context(tc.tile_pool(name="bat", bufs=2))
    lnc_p = ctx.enter_context(tc.tile_pool(name="lnc", bufs=1))

    for ch in range(NCH):
        tsl = slice(ch * TC, (ch + 1) * TC)

        # ---- LayerNorm apply (chunked): xnT[k] = ((xT - mu) * rstd) * g + b
        mus = lnc_p.tile([P, 2 * TC], FP32, tag="mus")
        nc.sync.dma_start(
            mus[:, 0:TC],
            mu_d.ap().rearrange("l one -> one l")[0:1, tsl].partition_broadcast(P))
        nc.sync.dma_start(
            mus[:, TC:2 * TC],
            rstd_d.ap().rearrange("l one -> one l")[0:1, tsl].partition_broadcast(P))
        for k in range(NDM):
            xtT = ev_p.tile([P, TC], FP32, tag="softp")
            nc.sync.dma_start(xtT[:], io["x_dl"][k * P:(k + 1) * P, tsl])
            nc.vector.tensor_sub(xtT[:], xtT[:], mus[:, 0:TC])
            nc.vector.tensor_mul(xtT[:], xtT[:], mus[:, TC:2 * TC])
            nc.scalar.activation(xnT[:, k * TC:(k + 1) * TC], xtT[:],
                                 ACTF.Identity, bias=b_c(k), scale=g_c(k))

        # ---- in_proj: xi half -> xz_xi store; z half -> silu'd later via DRAM
        zb = None
        for m in range(2 * NDH):
            wt = wi_p.tile([P, NDM * P], BF16, tag="w_in")
            nc.sync.dma_start(
                wt[:], io["in_w_pk"][:, m * NDM * P:(m + 1) * NDM * P])
            ps = mm_ps.tile([P, TC], FP32, tag="mm")
            for k in range(NDM):
                nc.tensor.matmul(ps[:], wt[:, k * P:(k + 1) * P],
                                 xnT[:, k * TC:(k + 1) * TC],
                                 start=(k == 0), stop=(k == NDM - 1))
            if m < NDH:
                nc.scalar.copy(xz_xi[:, m * CW + DC - 1:m * CW + DC - 1 + TC], ps[:])
            else:
                # store silu(z) = z * sigmoid(z); batch sigmoid over 4 m-tiles
                # to keep the ACT table from thrashing between exp and sigmoid
                j = (m - NDH) % G4
                if j == 0:
                    zb = bat_p.tile([P, G4, TC], BF16, tag="bat")
                nc.scalar.copy(zb[:, j, :], ps[:])
                if j == G4 - 1:
                    g4 = (m - NDH) // G4
                    zg = bat_p.tile([P, G4, TC], BF16, tag="sig")
                    nc.scalar.activation(zg[:], zb[:], ACTF.Sigmoid)
                    sz = bat_p.tile([P, G4, TC], BF16, tag="slu")
                    nc.vector.tensor_mul(sz[:], zb[:], zg[:])
                    nc.sync.dma_start(
                        z_d[g4 * G4 * P:(g4 + 1) * G4 * P, tsl]
                        .rearrange("(j p) t -> p j t", p=P), sz[:])

        # ---- causal depthwise conv (PE diag matmuls) + bias + silu -> xiT
        cvb = None
        for k in range(NDH):
            dg = wi_p.tile([P, DC * P], BF16, tag="w_dg")
            nc.sync.dma_start(
                dg[:], io["conv_dg_pk"][:, k * DC * P:(k + 1) * DC * P])
            ps = mm_ps.tile([P, TC], FP32, tag="mm")
            for t in range(DC):
                nc.tensor.matmul(
                    ps[:], dg[:, t * P:(t + 1) * P],
                    xz_xi[:, k * CW + t:k * CW + t + TC],
                    start=(t == 0), stop=(t == DC - 1))
            cv = ev_p.tile([P, TC], BF16, tag="cv")
            nc.scalar.activation(cv[:], ps[:], ACTF.Identity,
                                 bias=conv_b_c(k))
            cg = ev_p.tile([P, TC], BF16, tag="cg")
            nc.scalar.activation(cg[:], ps[:], ACTF.Sigmoid,
                                 bias=conv_b_c(k))
            nc.vector.tensor_mul(xiT[:, k * TC:(k + 1) * TC], cv[:], cg[:])
            # carry last DC-1 input cols to the head for the next chunk
            if ch + 1 < NCH:
                nc.vector.tensor_copy(xz_xi[:, k * CW:k * CW + DC - 1],
                                      xz_xi[:, k * CW + TC:k * CW + TC + DC - 1])

        # ---- x_proj -> proj[NPJ, TC] -> dtT, B, C
        psx = mm_ps.tile([NPJ, TC], FP32, tag="mm")
        for k in range(NDH):
            nc.tensor.matmul(psx[:], xprj_sb[:, k * NPJ:(k + 1) * NPJ],
                             xiT[:, k * TC:(k + 1) * TC],
                             start=(k == 0), stop=(k == NDH - 1))
        dtT = bat_p.tile([DTR, TC], BF16, tag="dtT")
        nc.scalar.copy(dtT[:], psx[0:DTR, :])
        bc_sb = bat_p.tile([2 * DS, TC], BF16, tag="bc")
        nc.scalar.copy(bc_sb[:], psx[DTR:NPJ, :])
        nc.sync.dma_start(bc_d[:, tsl], bc_sb[:])
        nc.sync.dma_start(b_big[:, :, :],
                          bc_d[0:DS, tsl].partition_broadcast(P))
        nc.sync.dma_start(c_big[:, :, :],
                          bc_d[DS:2 * DS, tsl].partition_broadcast(P))

        # ---- dt_proj + softplus -> delta (bf16)
        # softplus(x) = ln(1 + exp(x)); x <= ~2 here so exp never overflows
        for k in range(NDH):
            psd = mm_ps.tile([P, TC], FP32, tag="mm")
            nc.tensor.matmul(psd[:], dtw_sb[:, k * P:(k + 1) * P], dtT[:],
                             start=True, stop=True)
            ex = ev_p.tile([P, TC], BF16, tag="softp")
            nc.scalar.activation(ex[:], psd[:], ACTF.Exp, bias=dt_b_c(k))
            nc.scalar.activation(delta[:, k * TC:(k + 1) * TC], ex[:],
                                 ACTF.Ln, bias=1.0)

        # ---- selective scan per (d_tile, state) + y accumulation on PE
        for k in range(NDH):
            ksl = slice(k * TC, (k + 1) * TC)
            dx = t16_p.tile([P, TC], BF16, tag="dx")
            nc.vector.tensor_mul(dx[:], delta[:, ksl], xiT[:, ksl])
            xid = t16_p.tile([P, TC], BF16, tag="xid")
            nc.vector.tensor_scalar_mul(xid[:], xiT[:, ksl], d_c(k))
            yp = y_ps_p.tile([P, TC], FP32, tag="y")
            nc.tensor.matmul(yp[:], ident[:], xid[:], start=True, stop=False)
            for s in range(DS):
                av = sc_p.tile([P, TC], FP32, tag="a")
                nc.scalar.activation(av[:], delta[:, ksl], ACTF.Exp,
                                     scale=a_sb[:, k * DS + s:k * DS + s + 1])
                uv = sc_p.tile([P, TC], BF16, tag="u")
                if s < DS // 2:  # split u-multiplies across GPSIMD and DVE
                    nc.gpsimd.tensor_mul(uv[:], dx[:], b_big[:, s, :])
                else:
                    nc.vector.tensor_mul(uv[:], dx[:], b_big[:, s, :])
                init = 0.0 if ch == 0 else carry[:, k * DS + s:k * DS + s + 1]
                nc.vector.tensor_tensor_scan(h_big[:, s, :], av[:], uv[:],
                                             init, op0=OP.mult, op1=OP.add)
                if s % 2 == 1:  # hC for (s-1, s) in one paired multiply
                    hc = t16_p.tile([P, 2, TC], BF16, tag="hc")
                    nc.vector.tensor_mul(hc[:], h_big[:, s - 1:s + 1, :],
                                         c_big[:, s - 1:s + 1, :])
                    nc.tensor.matmul(yp[:], ident[:], hc[:, 0, :], start=False,
                                     stop=False)
                    nc.tensor.matmul(yp[:], ident[:], hc[:, 1, :], start=False,
                                     stop=(s == DS - 1))
            if ch + 1 < NCH:  # save carries h[:, -1] for all s of this d-tile
                nc.vector.tensor_copy(carry[:, k * DS:(k + 1) * DS],
                                      h_big[:, :, TC - 1])
            # gate: y * silu(z)  (z_d already holds silu(z))
            zl = t16_p.tile([P, TC], BF16, tag="zl")
            nc.sync.dma_start(zl[:], z_d[k * P:(k + 1) * P, tsl])
            nc.vector.tensor_mul(ygate[:, ksl], yp[:], zl[:])

        # ---- fused output projection
        for m in range(NDM):
            wt = wo_p.tile([P, NDH * P], BF16, tag="w_out")
            nc.sync.dma_start(
                wt[:], io["w_comb_pk"][:, m * NDH * P:(m + 1) * NDH * P])
            po = o_ps_p.tile([P, TC], FP32, tag="o")
            for k in range(NDH):
                nc.tensor.matmul(po[:], wt[:, k * P:(k + 1) * P],
                                 ygate[:, k * TC:(k + 1) * TC],
                                 start=(k == 0), stop=(k == NDH - 1))
            ot = ev_p.tile([P, TC], FP32, tag="out")
            nc.scalar.copy(ot[:], po[:])
            nc.sync.dma_start(out_d[m * P:(m + 1) * P, tsl], ot[:])


# ------------------------------------------------------------------
# host side
# ------------------------------------------------------------------

def _prep_core_inputs(cfg, xb, ln_g, ln_b, w):
    """xb: [L, DM] fp32 (already flipped for bwd). w: per-branch weights dict."""
    d = {
        "x_ld": np.ascontiguousarray(xb, np.float32),
        "x_dl": np.ascontiguousarray(xb.T, np.float32),
        "ln_g": np.ascontiguousarray(ln_g.reshape(-1, 1), np.float32),
        "ln_b": np.ascontiguousarray(ln_b.reshape(-1, 1), np.float32),
    }
    d.update(w)
    return d


def _prep_branch_weights(cfg, in_w, conv_w, conv_b, xproj_w, dt_w, dt_b,
                         A_log, D, out_w, merge_half):
    w_comb = merge_half.astype(np.float64) @ out_w.astype(np.float64)  # [DM, DI]
    P = cfg.P

    def pack_lhsT(w):  # w: [M, K] -> [P, (M//P)*K]; block m holds w[mP:(m+1)P].T
        M, Kd = w.shape
        blocks = [w[m * P:(m + 1) * P, :].reshape(P, Kd // P, P)
                  .transpose(2, 1, 0).reshape(P, Kd)
                  for m in range(M // P)]
        return np.ascontiguousarray(np.concatenate(blocks, axis=1), BF16_NP)

    cw = np.asarray(conv_w, np.float32)
    DI, DC = cw.shape
    dg = np.zeros((DI // P, DC, P, P), np.float32)
    idx = np.arange(P)
    for k in range(DI // P):
        for t in range(DC):
            dg[k, t, idx, idx] = cw[k * P:(k + 1) * P, t]
    dg_pk = np.ascontiguousarray(
        dg.transpose(2, 0, 1, 3).reshape(P, (DI // P) * DC * P), BF16_NP)

    return {
        "in_w_pk": pack_lhsT(np.asarray(in_w, np.float32)),
        "conv_dg_pk": dg_pk,
        "conv_w": np.ascontiguousarray(conv_w, np.float32),
        "conv_b": np.ascontiguousarray(conv_b.reshape(-1, 1), np.float32),
        "xproj_wT": np.ascontiguousarray(xproj_w.T, BF16_NP),
        "dt_wT": np.ascontiguousarray(dt_w.T, BF16_NP),
        "dt_b": np.ascontiguousarray(dt_b.reshape(-1, 1), np.float32),
        "A_neg": np.ascontiguousarray(-np.exp(A_log), np.float32),
        "D_vec": np.ascontiguousarray(D.reshape(-1, 1), np.float32),
        "w_comb_pk": pack_lhsT(w_comb.astype(np.float32)),
    }


_PROG_CACHE = {}


def _get_program(cfg: Cfg, num_devices: int):
    key = (cfg.L, cfg.DM, cfg.DI, cfg.DS, cfg.DTR, cfg.DC, cfg.TC, num_devices)
    if key not in _PROG_CACHE:
        _PROG_CACHE[key] = build_program(cfg, num_devices)
    return _PROG_CACHE[key]


def kernel(x, ln_g, ln_b, merge_w, merge_b,
           fwd_in_w, fwd_conv_w, fwd_conv_b, fwd_xproj_w, fwd_dt_w, fwd_dt_b,
           fwd_A_log, fwd_D, fwd_out_w,
           bwd_in_w, bwd_conv_w, bwd_conv_b, bwd_xproj_w, bwd_dt_w, bwd_dt_b,
           bwd_A_log, bwd_D, bwd_out_w):
    cfg = FULL
    x = np.asarray(x, np.float32)
    B = x.shape[0]
    assert x.shape == (B, cfg.L, cfg.DM) and B == 4

    nc = _get_program(cfg, 8)

    fw = _prep_branch_weights(cfg, fwd_in_w, fwd_conv_w, fwd_conv_b,
                              fwd_xproj_w, fwd_dt_w, fwd_dt_b, fwd_A_log,
                              fwd_D, fwd_out_w, np.asarray(merge_w)[:, :cfg.DM])
    bw = _prep_branch_weights(cfg, bwd_in_w, bwd_conv_w, bwd_conv_b,
                              bwd_xproj_w, bwd_dt_w, bwd_dt_b, bwd_A_log,
                              bwd_D, bwd_out_w, np.asarray(merge_w)[:, cfg.DM:])

    in_maps = []
    for c in range(8):
        br, b = divmod(c, 4)
        xb = x[b] if br == 0 else x[b, ::-1]
        in_maps.append(_prep_core_inputs(cfg, xb, np.asarray(ln_g),
                                         np.asarray(ln_b), fw if br == 0 else bw))

    global _last_in_maps
    _last_in_maps = in_maps
    res = run_bass_kernel_spmd(nc, in_maps, list(range(8)))
    parts = [r["part_out"] for r in res.results]  # [DM, L] each

    out = x.copy()
    for b in range(4):
        out[b] += parts[b].T
        out[b] += parts[4 + b].T[::-1]
    out += np.asarray(merge_b, np.float32)
    return out
